# revision 3
# baseline (speedup 1.0000x reference)
"""Trainium2 Bass kernel for nn_Model_47107201302874 (p-major streaming).

loss = sum((phi - lam)**2) with phi = kron(v_0..v_25), v_i = [sin|th_i|, cos|th_i|].

Sharding: core d owns the 2^23 lam elements whose top-3 bits equal d.
Locally, with p = bits 3..9 (128), k = bits 10..16 (128), s = bits 17..25 (512):
  phi[p, k, s] = A'[p] * B1[k] * B2[s],  A' = c_d * kron(v_3..v_9)
and the local loss splits exactly as
  sum(lam^2) - 2*sum(phi*lam) + sum(phi^2).

Streaming layout: lam viewed [128 p, 65536 (k s)] — p is the DRAM-major axis,
so each DMA line is a fully contiguous (chunk_width*4)-byte run per partition.
Per 512-wide subtile j (= one k):
  lam^2  : square+accumulate on one of DVE/ACT/POOL (round-robin) -> acc[:, j]
  cross  : PE matmul w[1,512] += ab1[:, j].T @ lam_subtile (float32r = full
           PE speed at 512 free, no bf16 cast needed)
Epilogue reduces acc on-chip to a single [1,1] scalar (PE ones-matmul for the
partition reduction) so the output DMA is one 4-byte descriptor.
"""

import os
import sys
from contextlib import ExitStack

import numpy as np

for _p in (
    "/opt/trn_rl_repo",
    "/root/.axon_site/_ro/trn_rl_repo",
    "/root/.axon_site/_ro/pypackages",
):
    if os.path.isdir(_p) and _p not in sys.path:
        sys.path.append(_p)

import concourse.bacc as bacc
import concourse.mybir as mybir
import concourse.tile as tile
from concourse.bass_utils import run_bass_kernel_spmd

F32 = mybir.dt.float32
F32R = mybir.dt.float32r
ALU = mybir.AluOpType
ACTF = mybir.ActivationFunctionType

N = 26
NCORES = 8
P = 128  # partitions: p bits 3..9
KS = 65536  # free: k,s bits 10..25
SUB = 512  # subtile width (one k value)
NSUB = 128
# chunk sizes in subtiles: small first chunk so compute starts early, 4 MiB
# steady-state chunks (32 KiB DMA lines), small tail chunks so the
# last-byte -> last-compute window is short.
CHUNK_SUBS = [4] + [8] * 14 + [4, 4, 2, 1, 1]
assert sum(CHUNK_SUBS) == NSUB
LAM_BUFS = 9
PRE_REDUCE_AT = 120  # reduce acc[:, :120] as soon as those columns are done
TAIL_SPLIT = 124  # last subtiles: split lam^2 across DVE/ACT halves
HALF = 288  # DVE half width (DVE is faster per element than ACT+accum-read)
# engine assignment pattern for the lam^2 square-accum (V=DVE, A=ACT).
# walrus rejects TensorScalarPtr on Pool, so the split is DVE/ACT only,
# ~time-balanced (DVE ~0.77us vs ACT ~1.0us per subtile). The tail strictly
# alternates (ending on V) so the last columns land with minimal serial lag.
SQ_PATTERN = "VAVAVAV"


# (A POOL tensor_tensor offload was tried here and reverted: gpsimd
# InstTensorTensor measures ~1.4-1.9us per [128,512] op on HW and its serial
# accumulator chain stalls lam buffer recycling.)
def sq_engine(j):
    if j >= PRE_REDUCE_AT:
        return "V" if (NSUB - 1 - j) % 2 == 0 else "A"
    return SQ_PATTERN[j % len(SQ_PATTERN)]
PI = float(np.pi)

_CACHE = {}


def _body(ctx, tc, out_ap, theta_ap, dbits_ap, lam_ap):
    nc = tc.nc
    const = ctx.enter_context(tc.tile_pool(name="const", bufs=1))
    psum_mm = ctx.enter_context(tc.tile_pool(name="psum_mm", bufs=1, space="PSUM"))
    psum_w = ctx.enter_context(tc.tile_pool(name="psum_w", bufs=1, space="PSUM"))
    lam_pool = ctx.enter_context(tc.tile_pool(name="lam", bufs=LAM_BUFS))

    # ---- prologue: per-factor sin/cos ------------------------------------
    # gpsimd (SWDGE) for the tiny loads keeps the sync HWDGE queue free to
    # start streaming lam immediately.
    th = const.tile([1, N], F32, tag="th")
    nc.gpsimd.dma_start(th[:], theta_ap)
    db = const.tile([1, 3], F32, tag="db")
    nc.gpsimd.dma_start(db[:], dbits_ap)

    av = const.tile([1, N], F32, tag="av")
    nc.scalar.activation(av[:], th[:], ACTF.Abs)

    # Sin LUT only valid on [-pi, pi]: wrap x (in [0, 3pi)) to x - 2pi*(x > pi).
    sn = const.tile([1, N], F32, tag="sn")
    cs = const.tile([1, N], F32, tag="cs")
    wa = const.tile([1, N], F32, tag="wa")
    wm = const.tile([1, N], F32, tag="wm")
    for dst, shift in ((sn, 0.0), (cs, PI / 2)):
        # wa = |th| + shift ; wm = (wa > pi) ; wa -= 2pi*wm ; dst = Sin(wa)
        if shift:
            nc.vector.tensor_scalar_add(wa[:], av[:], shift)
        else:
            nc.vector.tensor_copy(wa[:], av[:])
        nc.vector.tensor_scalar(wm[:], wa[:], PI, None, op0=ALU.is_gt)
        nc.vector.scalar_tensor_tensor(
            wa[:], wm[:], -2.0 * PI, wa[:], op0=ALU.mult, op1=ALU.add
        )
        nc.scalar.activation(dst[:], wa[:], ACTF.Sin)

    # c_d = prod_i (sn[i] + dbits[i]*(cs[i]-sn[i])), i<3
    sel = const.tile([1, 3], F32, tag="sel")
    nc.vector.tensor_sub(sel[:], cs[0:1, 0:3], sn[0:1, 0:3])
    nc.vector.tensor_mul(sel[:], sel[:], db[:])
    nc.vector.tensor_add(sel[:], sel[:], sn[0:1, 0:3])
    cd = const.tile([1, 1], F32, tag="cd")
    nc.vector.tensor_mul(cd[:], sel[0:1, 0:1], sel[0:1, 1:2])
    nc.vector.tensor_mul(cd[:], cd[:], sel[0:1, 2:3])

    # ---- kron ladders (free dim of partition 0) --------------------------
    kr_a = const.tile([1, SUB], F32, tag="kr_a")
    kr_b = const.tile([1, SUB], F32, tag="kr_b")

    def kron(idxs, seed, eng):
        # ladder on `eng`: DVE uses tensor_scalar_mul, ACT uses Copy+scale —
        # splitting the chains across engines shortens the DVE prologue.
        cur, other = kr_a, kr_b

        def mul(dst, src, L, sc):
            if eng == "act":
                nc.scalar.activation(dst, src[0:1, 0:L], ACTF.Copy, scale=sc)
            else:
                nc.vector.tensor_scalar_mul(dst, src[0:1, 0:L], sc)

        if seed is None:
            nc.vector.memset(cur[0:1, 0:1], 1.0)
        else:
            nc.vector.tensor_copy(cur[0:1, 0:1], seed)
        L = 1
        for i in idxs:
            d3 = other[0:1, 0 : 2 * L].rearrange("a (l t) -> a l t", t=2)
            mul(d3[:, :, 0], cur, L, sn[0:1, i : i + 1])
            mul(d3[:, :, 1], cur, L, cs[0:1, i : i + 1])
            cur, other = other, cur
            L *= 2
        return cur[0:1, 0:L]

    arow_src = kron(range(3, 10), cd, "act")  # [1,128] = c_d * A
    arow = const.tile([1, P], F32, tag="arow")
    nc.vector.tensor_copy(arow[:], arow_src)

    b1row_src = kron(range(10, 17), None, "dve")  # [1,128]
    b1row = const.tile([1, P], F32, tag="b1row")
    nc.vector.tensor_copy(b1row[:], b1row_src)

    # ab1[p, k] = A'[p] * B1[k] via outer-product matmul, kept fp32 (used as
    # float32r by the cross-term matmuls).
    ab1_ps = psum_mm.tile([P, P], F32, tag="ab1_ps")
    nc.tensor.matmul(ab1_ps[:], lhsT=arow[:], rhs=b1row[:], start=True, stop=True)
    ab1 = const.tile([P, P], F32R, tag="ab1")
    nc.scalar.copy(ab1[:], ab1_ps[:])

    b2row_src = kron(range(17, 26), None, "dve")  # [1,512]
    b2row = const.tile([1, SUB], F32, tag="b2row")
    nc.vector.tensor_copy(b2row[:], b2row_src)

    # phi^2 partial = sum(arow^2) * sum(b1row^2) * sum(b2row^2)
    phi2 = const.tile([1, 1], F32, tag="phi2")
    p2t = const.tile([1, SUB], F32, tag="p2t")
    p2s = const.tile([1, 1], F32, tag="p2s")
    nc.vector.memset(phi2[:], 1.0)
    for row, ln in ((arow, P), (b1row, P), (b2row, SUB)):
        nc.vector.tensor_mul(p2t[0:1, 0:ln], row[0:1, 0:ln], row[0:1, 0:ln])
        nc.vector.tensor_reduce(
            p2s[:], p2t[0:1, 0:ln], axis=mybir.AxisListType.X, op=ALU.add
        )
        nc.vector.tensor_mul(phi2[:], phi2[:], p2s[:])

    ones_col = const.tile([P, 1], F32, tag="ones_col")
    nc.vector.memset(ones_col[:], 1.0)

    # ---- main loop -------------------------------------------------------
    acc = const.tile([P, NSUB], F32, tag="acc")
    acc_b = const.tile([P, NSUB - TAIL_SPLIT], F32, tag="acc_b")
    scr_v = const.tile([P, SUB], F32, tag="scr_v")
    scr_a = const.tile([P, SUB], F32, tag="scr_a")
    w_ps = psum_w.tile([1, SUB], F32, tag="w_ps")

    j = 0
    for subs in CHUNK_SUBS:
        lt = lam_pool.tile([P, subs * SUB], F32R, tag="lt")
        nc.sync.dma_start(lt[:], lam_ap[:, j * SUB : (j + subs) * SUB].bitcast(F32R))
        for i in range(subs):
            slr = lt[:, i * SUB : (i + 1) * SUB]
            sl = slr.bitcast(F32)
            # cross: w[s] += sum_p ab1[p, j] * lam[p, j, s]
            nc.tensor.matmul(
                w_ps[:],
                lhsT=ab1[:, j : j + 1],
                rhs=slr,
                start=(j == 0),
                stop=(j == NSUB - 1),
            )
            # lam^2: square + accumulate into acc[:, j]
            if j >= TAIL_SPLIT:
                # split across both engines so the last columns land fast
                nc.vector.scalar_tensor_tensor(
                    scr_v[:, 0:HALF], sl[:, 0:HALF], 1.0, sl[:, 0:HALF],
                    op0=ALU.mult, op1=ALU.mult, accum_out=acc[:, j : j + 1],
                )
                nc.scalar.activation(
                    scr_a[:, 0 : SUB - HALF], sl[:, HALF:SUB], ACTF.Square,
                    accum_out=acc_b[:, j - TAIL_SPLIT : j - TAIL_SPLIT + 1],
                )
            elif sq_engine(j) == "V":
                nc.vector.scalar_tensor_tensor(
                    scr_v[:], sl, 1.0, sl, op0=ALU.mult, op1=ALU.mult,
                    accum_out=acc[:, j : j + 1],
                )
            else:
                nc.scalar.activation(
                    scr_a[:], sl, ACTF.Square, accum_out=acc[:, j : j + 1]
                )
            j += 1
        if j == PRE_REDUCE_AT:
            rsum_main = const.tile([P, 1], F32, tag="rsum_main")
            nc.vector.tensor_reduce(
                rsum_main[:], acc[:, 0:PRE_REDUCE_AT],
                axis=mybir.AxisListType.X, op=ALU.add,
            )


    # ---- epilogue --------------------------------------------------------
    # cross = sum_s w[s] * B2[s]: single fused stt with accumulator, reading
    # w straight from PSUM
    cm = const.tile([1, SUB], F32, tag="cm")
    cross = const.tile([1, 1], F32, tag="cross")
    nc.vector.scalar_tensor_tensor(
        cm[:], w_ps[:], 1.0, b2row[:], op0=ALU.mult, op1=ALU.mult,
        accum_out=cross[:],
    )
    # extra = phi2 - 2*cross
    extra = const.tile([1, 1], F32, tag="extra")
    nc.vector.scalar_tensor_tensor(
        extra[:], cross[:], -2.0, phi2[:], op0=ALU.mult, op1=ALU.add
    )

    # rsum[p] = sum_j acc[p, j] (bulk pre-reduced mid-stream); total = sum_p
    # rsum[p] via ones matmul
    rsum_tail = const.tile([P, 1], F32, tag="rsum_tail")
    nc.vector.tensor_reduce(
        rsum_tail[:], acc[:, PRE_REDUCE_AT:NSUB],
        axis=mybir.AxisListType.X, op=ALU.add,
    )
    rsum_b = const.tile([P, 1], F32, tag="rsum_b")
    nc.vector.tensor_reduce(
        rsum_b[:], acc_b[:], axis=mybir.AxisListType.X, op=ALU.add
    )
    rsum = const.tile([P, 1], F32, tag="rsum")
    nc.vector.tensor_add(rsum[:], rsum_main[:], rsum_tail[:])
    nc.vector.tensor_add(rsum[:], rsum[:], rsum_b[:])
    tot_ps = psum_mm.tile([1, 1], F32, tag="tot_ps")
    nc.tensor.matmul(tot_ps[:], lhsT=rsum[:], rhs=ones_col[:], start=True, stop=True)
    out_sb = const.tile([1, 1], F32, tag="out_sb")
    nc.vector.tensor_add(out_sb[:], tot_ps[:], extra[:])
    # SWDGE for the 4-byte result: avoids queuing the HWDGE ring behind the
    # lam stream and completes its semaphore faster
    nc.gpsimd.dma_start(out_ap, out_sb[:])


def build_nc(reps=1, loop=False):
    key = ("nc", reps, loop)
    if key in _CACHE:
        return _CACHE[key]
    nc = bacc.Bacc(
        "TRN2", target_bir_lowering=False, debug=False, num_devices=NCORES
    )
    theta_ap = nc.dram_tensor("theta", [1, N], F32, kind="ExternalInput").ap()
    dbits_ap = nc.dram_tensor("dbits", [1, 3], F32, kind="ExternalInput").ap()
    lam_ap = nc.dram_tensor("lam", [P, KS], F32, kind="ExternalInput").ap()
    out_ap = nc.dram_tensor("partial", [1, 1], F32, kind="ExternalOutput").ap()
    with tile.TileContext(nc) as tc, ExitStack() as ctx:
        _body(ctx, tc, out_ap, theta_ap, dbits_ap, lam_ap)
    nc.compile()
    _CACHE[key] = nc
    return nc


def make_in_maps(theta, lam):
    theta = np.ascontiguousarray(np.asarray(theta, dtype=np.float32)).reshape(1, N)
    lam = np.ascontiguousarray(np.asarray(lam, dtype=np.float32)).reshape(
        NCORES, P, KS
    )
    in_maps = []
    for d in range(NCORES):
        bits = np.array(
            [[(d >> 2) & 1, (d >> 1) & 1, d & 1]], dtype=np.float32
        )
        in_maps.append({"theta": theta, "dbits": bits, "lam": lam[d]})
    return in_maps


def run(theta, lam, trace=False, **kwargs):
    nc = build_nc()
    in_maps = make_in_maps(theta, lam)
    res = run_bass_kernel_spmd(
        nc, in_maps, list(range(NCORES)), trace=trace, **kwargs
    )
    total = np.float64(0.0)
    for r in res.results:
        total += r["partial"].astype(np.float64).sum()
    return np.array(np.float32(total)), res


def kernel(theta, lam):
    out, _ = run(theta, lam)
    return out
